# revision 23
# baseline (speedup 1.0000x reference)
"""AttentionEncoder (LSTM + input-dim attention) Trainium2 Bass kernel.

Key identity: the attention logits are e[b,d] = ex[b,d] + (h.Wh + c.Wc + ba)[b]
-- the recurrent part is a per-row constant across d, so
softmax(e) == softmax(ex): the attention weights are time-invariant and
independent of the LSTM state. Therefore:
  - att[b,t,:] = softmax(ex)[b,:]  (computed host-side, tiled over t)
  - wi[b,t,:]  = a[b,:] * x[b,t,:] (precomputed host-side, fed transposed)
and the device kernel is just the 64-step LSTM recurrence:
  gates = wi_t @ W_ih.T + h @ W_hh.T (+b);  i,f,g,o ->  c,h update.

Sharding: data-parallel over batch, 32 batches per core on 8 cores;
weights replicated; full scan on-chip per core.

Device-side layout per core (BL=32):
  - gates matmul: out[M=32, N=512] slices of one [32, 2048] psum tile,
    lhsT = transposed activations [K=128, 32] (wiT precomputed on host;
    hT via PE transpose each step). float32r operands (single-pass fp32
    PE mode: 1 cycle/column vs strict fp32's 4, TF32-like multiply
    precision, fp32 accumulate). float32r forbids PE column tiling, so
    all outputs sit at partitions 0:32 and gate chunks span the free dim.
  - gate order f,i,g,o; wi-part matmuls are emitted first (they only
    need host inputs) so they overlap the previous step's LSTM tail, and
    h-part matmuls run f,i,g,o so the sigmoid/tanh ACT ops start while
    later gate chunks still accumulate.
  - everything elementwise sits at partition base 0 (walrus requires all
    SBUF operands of a DVE op to share the start partition): the c state
    and tanh(g) are free-dim halves of one [32, 1024] tile.
"""

import sys
import numpy as np

sys.path.insert(0, "/opt/trn_rl_repo")

import concourse.bacc as bacc
import concourse.mybir as mybir
from concourse.tile import TileContext
from concourse.bass_utils import run_bass_kernel_spmd

F32 = mybir.dt.float32
F32R = mybir.dt.float32r
AF = mybir.ActivationFunctionType
ALU = mybir.AluOpType

B, L, D, H = 256, 64, 256, 512
NCORES = 8
BL = B // NCORES  # 32 local batch
NG = 4  # gate chunks (f, i, o, g) of 512 each


def build_nc(has_bias: bool):
    nc = bacc.Bacc(None, target_bir_lowering=False)

    wiT_d = nc.dram_tensor("wiT", [128, L * 2 * 32], F32R, kind="ExternalInput")
    w_d = nc.dram_tensor("wmat", [128, 6 * NG * 512], F32R, kind="ExternalInput")
    ht0_d = nc.dram_tensor("ht0", [128, 128], F32R, kind="ExternalInput")
    c0_d = nc.dram_tensor("c0", [BL, H], F32, kind="ExternalInput")
    id_d = nc.dram_tensor("ident", [32, 32], F32, kind="ExternalInput")
    if has_bias:
        bias_d = nc.dram_tensor("bias", [1, 4 * H], F32, kind="ExternalInput")
    enc_d = nc.dram_tensor("enc", [BL, L, H], F32, kind="ExternalOutput")

    with TileContext(nc) as tc:
        with (
            tc.tile_pool(name="const", bufs=1) as cpool,
            tc.tile_pool(name="state", bufs=1) as spool,
            tc.tile_pool(name="work", bufs=4) as wpool,
            tc.tile_pool(name="gpsum", bufs=1, space="PSUM") as gpsum,
            tc.tile_pool(name="tpsum", bufs=2, space="PSUM") as tpsum,
        ):
            # ---- inputs resident in SBUF (one tile per DMA) ----
            wiq = []
            for i in range(4):
                q = (L * 2 * 32) // 4
                wt = cpool.tile([128, q], F32R, tag=f"wi{i}", name=f"wi{i}")
                nc.sync.dma_start(wt[:], wiT_d[:][:, i * q:(i + 1) * q])
                wiq.append(wt)
            wk = []
            for k in range(6):
                q = NG * 512
                wt = cpool.tile([128, q], F32R, tag=f"w{k}", name=f"w{k}")
                nc.sync.dma_start(wt[:], w_d[:][:, k * q:(k + 1) * q])
                wk.append(wt)
            id_sb = cpool.tile([32, 32], F32, tag="ident")
            nc.sync.dma_start(id_sb[:], id_d[:])
            if has_bias:
                bias_sb = cpool.tile([1, 4 * H], F32, tag="bias")
                nc.sync.dma_start(bias_sb[:], bias_d[:])
                ones_sb = cpool.tile([1, 32], F32, tag="ones")
                nc.vector.memset(ones_sb, 1.0)

            # ---- state (ping-pong) ----
            hT = [spool.tile([128, 128], F32R, tag=f"hT{i}", name=f"hT{i}")
                  for i in range(2)]
            cst = [spool.tile([32, H], F32, tag=f"c{i}", name=f"c{i}")
                   for i in range(2)]
            nc.sync.dma_start(hT[0][:], ht0_d[:])
            nc.sync.dma_start(cst[0][:], c0_d[:])

            def wi_chunk(t, c):
                # wiT slice for step t, contraction chunk c (0/1): [128, 32]
                i = t // 16
                off = (t % 16) * 64 + c * 32
                return wiq[i][:, off:off + 32]

            for t in range(L):
                cur, nxt = t % 2, (t + 1) % 2

                # ---------- gates ----------
                # four separate psum tiles (one per gate chunk: f,i,g,o)
                # so each ACT can fire as soon as its own bank closes.
                gb = [gpsum.tile([32, 512], F32, tag=f"gb{j}", name=f"gb{j}")
                      for j in range(NG)]
                for j in range(NG):
                    for k in range(2):  # wi part: overlaps previous tail
                        nc.tensor.matmul(
                            gb[j], wi_chunk(t, k),
                            wk[k][:, j * 512:(j + 1) * 512],
                            start=(k == 0), stop=False,
                            skip_group_check=True,
                        )
                    if has_bias:
                        nc.tensor.matmul(
                            gb[j], ones_sb, bias_sb[:, 512 * j:512 * (j + 1)],
                            start=False, stop=False,
                            skip_group_check=True,
                        )
                for j in range(NG):
                    for k in range(2, 6):  # h part
                        nc.tensor.matmul(
                            gb[j],
                            hT[cur][:, 32 * (k - 2):32 * (k - 2) + 32],
                            wk[k][:, j * 512:(j + 1) * 512],
                            start=False, stop=(k == 5),
                            skip_group_check=True,
                        )

                # ---------- LSTM elementwise ----------
                g_t = wpool.tile([32, 512], F32, tag="g_t")
                nc.scalar.activation(g_t, gb[0], AF.Tanh)
                sig_f = wpool.tile([32, 512], F32, tag="sig_f")
                nc.scalar.activation(sig_f, gb[1], AF.Sigmoid)
                sig_i = wpool.tile([32, 512], F32, tag="sig_i")
                nc.scalar.activation(sig_i, gb[2], AF.Sigmoid)
                sig_o = wpool.tile([32, 512], F32, tag="sig_o")
                nc.scalar.activation(sig_o, gb[3], AF.Sigmoid)
                prod_f = wpool.tile([32, 512], F32, tag="prod_f")
                nc.vector.tensor_mul(prod_f, sig_f, cst[cur])
                prod_i = wpool.tile([32, 512], F32, tag="prod_i")
                nc.vector.tensor_mul(prod_i, sig_i, g_t)
                # halved tail: c_new -> tanh -> h per 256-col half, split
                # across DVE (half 0) and GPSIMD (half 1) to run concurrently
                tc_sb = wpool.tile([32, 512], F32, tag="tc")
                h_sb = wpool.tile([32, 512], F32, tag="h")
                for hf in range(2):
                    sl = slice(256 * hf, 256 * (hf + 1))
                    eng = nc.vector if hf == 0 else nc.gpsimd
                    eng.tensor_add(cst[nxt][:, sl], prod_f[:, sl], prod_i[:, sl])
                    nc.scalar.activation(tc_sb[:, sl], cst[nxt][:, sl], AF.Tanh)
                    eng.tensor_mul(h_sb[:, sl], sig_o[:, sl], tc_sb[:, sl])
                nc.sync.dma_start(enc_d[:][:, t, :], h_sb[:])

                # ---------- transpose h for next step ----------
                # per 256-col half so the copy (and next step's matmuls)
                # start as soon as that half of h is ready
                if t + 1 < L:
                    t_ps = tpsum.tile([128, 128], F32, tag="hT_ps")
                    for hf in range(2):
                        for k in range(2 * hf, 2 * hf + 2):
                            nc.tensor.transpose(
                                t_ps[:, 32 * k:32 * (k + 1)],
                                h_sb[:, 128 * k:128 * (k + 1)], id_sb)
                        nc.scalar.copy(hT[nxt][:, 64 * hf:64 * (hf + 1)],
                                       t_ps[:, 64 * hf:64 * (hf + 1)])

    nc.finalize()
    return nc


_CACHE = {}


def _get_nc(has_bias):
    if has_bias not in _CACHE:
        _CACHE[has_bias] = build_nc(has_bias)
    return _CACHE[has_bias]


def kernel(x, h0, c0, W_ih, W_hh, b_ih, b_hh, Wa, ba):
    x = np.asarray(x, np.float32)
    h0 = np.asarray(h0, np.float32)
    c0 = np.asarray(c0, np.float32)
    W_ih = np.asarray(W_ih, np.float32)
    W_hh = np.asarray(W_hh, np.float32)
    b_ih = np.asarray(b_ih, np.float32)
    b_hh = np.asarray(b_hh, np.float32)
    Wa = np.asarray(Wa, np.float32)

    Wx = Wa[2 * H:]
    bias = (b_ih + b_hh).astype(np.float32)
    has_bias = bool(np.any(bias))

    # attention: a = softmax(ex) -- time/state invariant (see module doc)
    ex = np.einsum("bld,l->bd", x.astype(np.float64), Wx.astype(np.float64))
    aw = np.exp(ex - ex.max(axis=1, keepdims=True))
    aw /= aw.sum(axis=1, keepdims=True)           # [B, D] float64
    att = np.broadcast_to(aw[:, None, :].astype(np.float32),
                          (B, L, D)).copy()
    wi = (aw[:, None, :] * x.astype(np.float64)).astype(np.float32)  # [B,L,D]

    # gate order i,f,g,o -> g,f,i,o: tanh(g) is deepest in the c_new
    # chain, so its bank accumulates first
    perm = np.concatenate([np.arange(2 * H, 3 * H), np.arange(H, 2 * H),
                           np.arange(0, H), np.arange(3 * H, 4 * H)])
    Wcomb = np.vstack([W_ih[perm].T, W_hh[perm].T]).astype(np.float32)  # [768,2048]
    bias_p = bias[perm][None, :].astype(np.float32)
    blocks = [Wcomb[128 * k:128 * (k + 1), 512 * j:512 * (j + 1)]
              for k in range(6) for j in range(NG)]
    wmat = np.ascontiguousarray(np.concatenate(blocks, axis=1))  # [128, 12288]

    ident = np.eye(32, dtype=np.float32)

    nc = _get_nc(has_bias)

    in_maps = []
    for cid in range(NCORES):
        sl = slice(cid * BL, (cid + 1) * BL)
        h0s, c0s = h0[sl], c0[sl]
        ht0 = np.transpose(h0s.T.reshape(4, 128, BL), (1, 0, 2)).reshape(128, 128)
        # wiT[p, t*64 + c*32 + b] = wi[b, t, 128c + p]
        wis = wi[sl]  # [32, 64, 256]
        wiT = np.transpose(wis.reshape(BL, L, 2, 128), (3, 1, 2, 0)) \
                .reshape(128, L * 64)
        m = {
            "wiT": np.ascontiguousarray(wiT),
            "wmat": wmat,
            "ht0": np.ascontiguousarray(ht0.astype(np.float32)),
            "c0": np.ascontiguousarray(c0s),
            "ident": ident,
        }
        if has_bias:
            m["bias"] = bias_p
        in_maps.append(m)

    res = run_bass_kernel_spmd(nc, in_maps, core_ids=list(range(NCORES)))
    enc = np.concatenate([r["enc"] for r in res.results], axis=0)
    return att, enc
